# revision 5
# baseline (speedup 1.0000x reference)
"""Trainium2 Bass kernel for the constrained Hamiltonian NN (CHNN) vector field.

Math: the reference solves, per batch sample,
    out = JdH - J DPhi X,   A X = DPhi^T JdH,   A = DPhi^T J DPhi  (64x64)
For chain constraints DPhi = [[Dr, Ddot], [0, Minv Dr]] and
    A = [[0, G], [-G, K]],  G = Dr^T Minv Dr  (32x32 tridiagonal SPD),
    K = Ddot^T Minv Dr - Dr^T Minv Ddot      (tridiagonal antisymmetric)
so X = [x0; x1] with x1 = G^{-1} b0 and x0 = G^{-1}(K x1 - b1), where
    b0 = Dr^T v,  b1 = Ddot^T v - Dr^T Minv g,  v = Minv p,  g = dV/dr.
Output: out_r = v - Minv Dr x1 ; out_p = -g + Dr x0 + Ddot x1.

The MLP grad g runs feature-major on the tensor engine (batch on the free
axis); the tridiagonal solves run batch-major on the vector engine via
parallel cyclic reduction (PCR, 5 levels), with per-level coefficients kept
so the second solve is a cheap rhs-only replay.

Sharding: pure data-parallel over the batch axis across 8 cores.
"""

import numpy as np

N, D = 32, 2
ZD = 128            # state dim
BS = 4096           # full batch
NCORES = 8
BSL = BS // NCORES  # 512 per core
C4 = BSL // 128     # 4 batch chunks of 128 partitions
AC = 16             # PCR active-window column offset (pad width = max shift)


def build_program(debug=False):
    """Build + compile the single-core SPMD Bass/Tile program."""
    from contextlib import ExitStack

    import concourse.bass as bass
    import concourse.mybir as mybir
    import concourse.tile as tile
    from concourse import bacc
    from concourse.masks import make_identity

    f32 = mybir.dt.float32
    AF = mybir.ActivationFunctionType
    OP = mybir.AluOpType

    nc = bacc.Bacc(
        "TRN2",
        target_bir_lowering=False,
        debug=debug,
        enable_asserts=True,
        num_devices=NCORES,
    )

    z = nc.dram_tensor("z", [BSL, ZD], f32, kind="ExternalInput")
    w0 = nc.dram_tensor("w0", [64, 256], f32, kind="ExternalInput")
    w1 = nc.dram_tensor("w1", [256, 256], f32, kind="ExternalInput")
    w2 = nc.dram_tensor("w2", [256, 256], f32, kind="ExternalInput")
    w1t = nc.dram_tensor("w1t", [256, 256], f32, kind="ExternalInput")
    w2t = nc.dram_tensor("w2t", [256, 256], f32, kind="ExternalInput")
    w0t = nc.dram_tensor("w0t", [256, 64], f32, kind="ExternalInput")
    bias0 = nc.dram_tensor("bias0", [256], f32, kind="ExternalInput")
    bias1 = nc.dram_tensor("bias1", [256], f32, kind="ExternalInput")
    bias2 = nc.dram_tensor("bias2", [256], f32, kind="ExternalInput")
    w3 = nc.dram_tensor("w3", [256], f32, kind="ExternalInput")
    cc = nc.dram_tensor("cc", [128, 512], f32, kind="ExternalInput")
    out = nc.dram_tensor("out", [BSL, ZD], f32, kind="ExternalOutput")

    with tile.TileContext(nc) as tc:
        with ExitStack() as ctx:
            const = ctx.enter_context(tc.tile_pool(name="const", bufs=1))
            main = ctx.enter_context(tc.tile_pool(name="main", bufs=1))
            scr = ctx.enter_context(tc.tile_pool(name="scr", bufs=2))
            psmm = ctx.enter_context(tc.tile_pool(name="psmm", bufs=4, space="PSUM"))
            pstr = ctx.enter_context(tc.tile_pool(name="pstr", bufs=2, space="PSUM"))

            # ---- constants ----
            w0sb = const.tile([64, 256], f32)
            nc.sync.dma_start(out=w0sb, in_=w0.ap())
            w1sb = const.tile([128, 2, 256], f32)
            nc.sync.dma_start(out=w1sb, in_=w1.ap().rearrange("(k p) n -> p k n", p=128))
            w2sb = const.tile([128, 2, 256], f32)
            nc.sync.dma_start(out=w2sb, in_=w2.ap().rearrange("(k p) n -> p k n", p=128))
            w1tsb = const.tile([128, 2, 256], f32)
            nc.sync.dma_start(out=w1tsb, in_=w1t.ap().rearrange("(k p) n -> p k n", p=128))
            w2tsb = const.tile([128, 2, 256], f32)
            nc.sync.dma_start(out=w2tsb, in_=w2t.ap().rearrange("(k p) n -> p k n", p=128))
            w0tsb = const.tile([128, 2, 64], f32)
            nc.sync.dma_start(out=w0tsb, in_=w0t.ap().rearrange("(k p) n -> p k n", p=128))
            b0sb = const.tile([128, 2], f32)
            nc.sync.dma_start(out=b0sb, in_=bias0.ap().rearrange("(c p) -> p c", p=128))
            b1sb = const.tile([128, 2], f32)
            nc.sync.dma_start(out=b1sb, in_=bias1.ap().rearrange("(c p) -> p c", p=128))
            b2sb = const.tile([128, 2], f32)
            nc.sync.dma_start(out=b2sb, in_=bias2.ap().rearrange("(c p) -> p c", p=128))
            w3sb = const.tile([128, 2], f32)
            nc.sync.dma_start(out=w3sb, in_=w3.ap().rearrange("(c p) -> p c", p=128))
            ccsb = const.tile([128, 512], f32)
            nc.sync.dma_start(out=ccsb, in_=cc.ap())
            invmP = ccsb[:, 0:256].rearrange("p (c f) -> p c f", f=64)
            jnvP = ccsb[:, 256:384].rearrange("p (c f) -> p c f", f=32)
            ecoP = ccsb[:, 384:512].rearrange("p (c f) -> p c f", f=32)
            ident = const.tile([128, 128], f32)
            make_identity(nc, ident)

            # ---- load z batch-major: [p, chunk, feature] ----
            zt = main.tile([128, C4, ZD], f32)
            nc.sync.dma_start(out=zt, in_=z.ap().rearrange("(c p) f -> p c f", p=128))

            # ---- chain quantities (batch-major), independent of the MLP ----
            vt = main.tile([128, C4, 64], f32)       # v = Minv p
            nc.vector.tensor_mul(vt, zt[:, :, 64:128], invmP)
            ut = main.tile([128, C4, 64], f32)       # u~ = diff(r) (u = 2 u~)
            nc.vector.tensor_sub(ut[:, :, 2:64], zt[:, :, 0:62], zt[:, :, 2:64])
            nc.gpsimd.tensor_copy(ut[:, :, 0:2], zt[:, :, 0:2])
            wt = main.tile([128, C4, 64], f32)       # w~ = diff(v)
            nc.vector.tensor_sub(wt[:, :, 2:64], vt[:, :, 0:62], vt[:, :, 2:64])
            nc.gpsimd.tensor_copy(wt[:, :, 0:2], vt[:, :, 0:2])

            def prodpair(dst, xa, ya, npair):
                # dst[:, :, 0:npair] = pairwise-д sum of xa*ya (over D=2)
                pr = scr.tile([128, C4, 64], f32, tag="prod")
                nc.vector.tensor_mul(pr[:, :, 0 : 2 * npair], xa, ya)
                pe = pr.rearrange("p c (i d) -> p c i d", d=2)
                nc.vector.tensor_add(
                    dst[:, :, 0:npair], pe[:, :, 0:npair, 0], pe[:, :, 0:npair, 1]
                )

            dcc = main.tile([128, C4, 32], f32)
            prodpair(dcc, ut, ut, 32)
            puw = main.tile([128, C4, 32], f32)
            prodpair(puw, ut, wt, 32)
            pww = main.tile([128, C4, 32], f32)
            prodpair(pww, wt, wt, 32)
            cuu = main.tile([128, C4, 32], f32)
            prodpair(cuu, ut[:, :, 0:62], ut[:, :, 2:64], 31)
            cwu = main.tile([128, C4, 32], f32)
            prodpair(cwu, wt[:, :, 0:62], ut[:, :, 2:64], 31)
            cuw = main.tile([128, C4, 32], f32)
            prodpair(cuw, ut[:, :, 0:62], wt[:, :, 2:64], 31)

            # ---- PCR system assembly (padded tiles, active cols AC..AC+32) ----
            apcr = main.tile([128, C4, 64], f32)
            nc.gpsimd.memset(apcr, 1.0)
            et = [main.tile([128, C4, 64], f32, tag=f"et{i}", name=f"et{i}") for i in range(2)]
            bt = [main.tile([128, C4, 64], f32, tag=f"bt{i}", name=f"bt{i}") for i in range(2)]
            for tl_ in (*et, *bt):
                nc.gpsimd.memset(tl_, 0.0)
            kkp = main.tile([128, C4, 64], f32)
            nc.gpsimd.memset(kkp, 0.0)
            x1p = main.tile([128, C4, 64], f32)
            nc.gpsimd.memset(x1p, 0.0)
            x0p = main.tile([128, C4, 64], f32)
            nc.gpsimd.memset(x0p, 0.0)

            def A_(t, off=0, w=32):
                return t[:, :, AC + off : AC + off + w]

            nc.vector.tensor_mul(A_(apcr), jnvP, dcc)
            nc.vector.tensor_mul(A_(et[0], 0, 31), ecoP[:, :, 0:31], cuu[:, :, 0:31])
            nc.vector.tensor_scalar_mul(A_(bt[0]), puw, 2.0)
            cdt = scr.tile([128, C4, 32], f32, tag="cdt")
            nc.vector.tensor_sub(cdt[:, :, 0:31], cwu[:, :, 0:31], cuw[:, :, 0:31])
            nc.vector.tensor_mul(A_(kkp, 0, 31), ecoP[:, :, 0:31], cdt[:, :, 0:31])

            # ---- PCR factorization + solve G x1 = b0 ----
            rt_ = main.tile([128, C4, 64], f32)
            nrt = main.tile([128, C4, 64], f32)
            tl = [main.tile([128, C4, 64], f32, tag=f"tl{i}", name=f"tl{i}") for i in range(5)]
            bnl = [main.tile([128, C4, 32], f32, tag=f"bn{i}", name=f"bn{i}") for i in range(5)]
            shifts = [1, 2, 4, 8, 16]
            cur = 0
            for lev, s in enumerate(shifts):
                eC, eN = et[cur], et[1 - cur]
                bC, bN = bt[cur], bt[1 - cur]
                nc.vector.reciprocal(rt_, apcr)
                nc.vector.tensor_scalar_mul(nrt, rt_, -1.0)
                nc.vector.tensor_mul(tl[lev], eC, nrt)
                nc.vector.tensor_mul(bnl[lev], A_(eC), A_(nrt, s))
                t2 = scr.tile([128, C4, 64], f32, tag="t2")
                nc.vector.tensor_mul(t2, tl[lev], eC)
                u2 = scr.tile([128, C4, 32], f32, tag="u2")
                nc.vector.tensor_mul(u2, bnl[lev], A_(eC))
                nc.vector.tensor_add(A_(apcr), A_(apcr), A_(t2, -s))
                nc.vector.tensor_add(A_(apcr), A_(apcr), u2)
                tb = scr.tile([128, C4, 64], f32, tag="tb")
                nc.vector.tensor_mul(tb, tl[lev], bC)
                m2 = scr.tile([128, C4, 32], f32, tag="m2")
                nc.vector.tensor_mul(m2, bnl[lev], A_(bC, s))
                nc.vector.tensor_add(A_(bN), A_(bC), A_(tb, -s))
                nc.vector.tensor_add(A_(bN), A_(bN), m2)
                nc.vector.tensor_mul(A_(eN), bnl[lev], A_(eC, s))
                cur = 1 - cur
            nc.vector.reciprocal(rt_, apcr)
            nc.vector.tensor_mul(A_(x1p), A_(bt[cur]), A_(rt_))

            # ---- MLP: g = dV/dr, feature-major [feature, batch] ----
            rT = main.tile([64, BSL], f32)
            for c in range(C4):
                pt = pstr.tile([64, 128], f32, tag="ptr")
                nc.tensor.transpose(pt, zt[:, c, 0:64], ident)
                nc.vector.tensor_copy(rT[:, c * 128 : (c + 1) * 128], pt)

            # softplus h = Ln(Exp(x)+1), sigmoid s = Exp(x - h); all ACT ops
            # stay in the natural_log_exp activation table (no table reloads)
            def act_pair(ps, bsl, m, h_dst, s_dst):
                tE = scr.tile([128, BSL], f32, tag="tE")
                nc.scalar.activation(tE, ps, AF.Exp, bias=bsl[:, m : m + 1])
                nc.scalar.activation(h_dst, tE, AF.Ln, bias=1.0)
                dq = scr.tile([128, BSL], f32, tag="dq")
                nc.vector.scalar_tensor_tensor(
                    dq, ps, bsl[:, m : m + 1], h_dst,
                    op0=OP.add, op1=OP.subtract,
                )
                nc.scalar.activation(s_dst, dq, AF.Exp)

            h0 = main.tile([128, 2, BSL], f32)
            s0 = main.tile([128, 2, BSL], f32)
            for m in range(2):
                ps = psmm.tile([128, BSL], f32, tag="mm")
                nc.tensor.matmul(
                    ps, w0sb[:, m * 128 : (m + 1) * 128], rT, start=True, stop=True
                )
                act_pair(ps, b0sb, m, h0[:, m, :], s0[:, m, :])
            h1 = main.tile([128, 2, BSL], f32)
            s1 = main.tile([128, 2, BSL], f32)
            for m in range(2):
                ps = psmm.tile([128, BSL], f32, tag="mm")
                for k in range(2):
                    nc.tensor.matmul(
                        ps,
                        w1sb[:, k, m * 128 : (m + 1) * 128],
                        h0[:, k, :],
                        start=(k == 0),
                        stop=(k == 1),
                    )
                act_pair(ps, b1sb, m, h1[:, m, :], s1[:, m, :])
            dp2 = main.tile([128, 2, BSL], f32)
            for m in range(2):
                ps = psmm.tile([128, BSL], f32, tag="mm")
                for k in range(2):
                    nc.tensor.matmul(
                        ps,
                        w2sb[:, k, m * 128 : (m + 1) * 128],
                        h1[:, k, :],
                        start=(k == 0),
                        stop=(k == 1),
                    )
                # layer 2: only the sigmoid is needed (h2 feeds V, unused)
                h2m = scr.tile([128, BSL], f32, tag="h2m")
                s2m = scr.tile([128, BSL], f32, tag="s2m")
                act_pair(ps, b2sb, m, h2m, s2m)
                nc.vector.tensor_scalar_mul(dp2[:, m, :], s2m, w3sb[:, m : m + 1])
            dp1 = main.tile([128, 2, BSL], f32)
            for m in range(2):
                ps = psmm.tile([128, BSL], f32, tag="mm")
                for k in range(2):
                    nc.tensor.matmul(
                        ps,
                        w2tsb[:, k, m * 128 : (m + 1) * 128],
                        dp2[:, k, :],
                        start=(k == 0),
                        stop=(k == 1),
                    )
                nc.vector.tensor_mul(dp1[:, m, :], ps, s1[:, m, :])
            dp0 = main.tile([128, 2, BSL], f32)
            for m in range(2):
                ps = psmm.tile([128, BSL], f32, tag="mm")
                for k in range(2):
                    nc.tensor.matmul(
                        ps,
                        w1tsb[:, k, m * 128 : (m + 1) * 128],
                        dp1[:, k, :],
                        start=(k == 0),
                        stop=(k == 1),
                    )
                nc.vector.tensor_mul(dp0[:, m, :], ps, s0[:, m, :])
            gps = psmm.tile([64, BSL], f32, tag="mm")
            for k in range(2):
                nc.tensor.matmul(
                    gps, w0tsb[:, k, :], dp0[:, k, :], start=(k == 0), stop=(k == 1)
                )
            gsb = main.tile([64, BSL], f32)
            nc.vector.tensor_copy(gsb, gps)
            gbm = main.tile([128, C4, 64], f32)   # g batch-major
            for c in range(C4):
                pt = pstr.tile([128, 64], f32, tag="ptg")
                nc.tensor.transpose(
                    pt, gsb[:, c * 128 : (c + 1) * 128], ident[0:64, 0:64]
                )
                nc.vector.tensor_copy(gbm[:, c, :], pt)

            # ---- b1 pieces that need g ----
            gt = main.tile([128, C4, 64], f32)    # Minv g
            nc.vector.tensor_mul(gt, gbm, invmP)
            gd = main.tile([128, C4, 64], f32)    # chain-diff of Minv g
            nc.vector.tensor_sub(gd[:, :, 2:64], gt[:, :, 0:62], gt[:, :, 2:64])
            nc.gpsimd.tensor_copy(gd[:, :, 0:2], gt[:, :, 0:2])
            pugd = main.tile([128, C4, 32], f32)
            prodpair(pugd, ut, gd, 32)
            b1t = main.tile([128, C4, 32], f32)   # (b1 = 2*b1t)
            nc.vector.tensor_sub(b1t, pww, pugd)

            # ---- rhs2 = K x1 - b1; solve G x0 = rhs2 by replaying PCR ----
            t1 = scr.tile([128, C4, 32], f32, tag="t1")
            nc.vector.tensor_mul(t1, A_(kkp), A_(x1p, 1))
            t2k = scr.tile([128, C4, 32], f32, tag="t2k")
            nc.vector.tensor_mul(t2k, A_(kkp, -1), A_(x1p, -1))
            ttk = scr.tile([128, C4, 32], f32, tag="ttk")
            nc.vector.tensor_sub(ttk, t1, t2k)
            nc.vector.scalar_tensor_tensor(
                A_(bt[0]), b1t, -2.0, ttk, op0=OP.mult, op1=OP.add
            )
            cur = 0
            for lev, s in enumerate(shifts):
                bC, bN = bt[cur], bt[1 - cur]
                tb = scr.tile([128, C4, 64], f32, tag="tb")
                nc.vector.tensor_mul(tb, tl[lev], bC)
                m2 = scr.tile([128, C4, 32], f32, tag="m2")
                nc.vector.tensor_mul(m2, bnl[lev], A_(bC, s))
                nc.vector.tensor_add(A_(bN), A_(bC), A_(tb, -s))
                nc.vector.tensor_add(A_(bN), A_(bN), m2)
                cur = 1 - cur
            nc.vector.tensor_mul(A_(x0p), A_(bt[cur]), A_(rt_))

            # ---- outputs ----
            # x?e: expand per-constraint x to per-feature (x2 scale folds u=2u~)
            x1e = main.tile([128, C4, 64], f32)
            x0e = main.tile([128, C4, 64], f32)
            for xe, xp in ((x1e, x1p), (x0e, x0p)):
                xv = xe.rearrange("p c (i d) -> p c i d", d=2)
                nc.gpsimd.tensor_scalar_mul(xv[:, :, :, 0], A_(xp), 2.0)
                nc.gpsimd.tensor_scalar_mul(xv[:, :, :, 1], A_(xp), 2.0)
            A1 = main.tile([128, C4, 64], f32)    # x1 * u  (per feature)
            nc.vector.tensor_mul(A1, x1e, ut)
            Bt1 = main.tile([128, C4, 64], f32)   # x0*u + x1*w
            nc.vector.tensor_mul(Bt1, x0e, ut)
            bs_ = scr.tile([128, C4, 64], f32, tag="bs_")
            nc.vector.tensor_mul(bs_, x1e, wt)
            nc.vector.tensor_add(Bt1, Bt1, bs_)

            outt = main.tile([128, C4, ZD], f32)

            def drx(dst, src):
                # dst = Dr-combine(src): node0: s0+s1; mid: s_{i+1}-s_i; last: -s
                nc.vector.tensor_add(dst[:, :, 0:2], src[:, :, 2:4], src[:, :, 0:2])
                nc.vector.tensor_sub(dst[:, :, 2:62], src[:, :, 4:64], src[:, :, 2:62])
                nc.vector.tensor_scalar_mul(dst[:, :, 62:64], src[:, :, 62:64], -1.0)

            drA = main.tile([128, C4, 64], f32)
            drx(drA, A1)
            sD = scr.tile([128, C4, 64], f32, tag="sD")
            nc.vector.tensor_mul(sD, drA, invmP)
            nc.vector.tensor_sub(outt[:, :, 0:64], vt, sD)
            drB = main.tile([128, C4, 64], f32)
            drx(drB, Bt1)
            nc.vector.tensor_sub(outt[:, :, 64:128], drB, gbm)

            nc.sync.dma_start(
                out=out.ap().rearrange("(c p) f -> p c f", p=128), in_=outt
            )

    nc.compile()
    return nc


def host_inputs(inputs):
    """Host-side prep: per-core input maps (weights replicated, z sharded)."""
    f = lambda x: np.ascontiguousarray(np.asarray(x, np.float32))
    z = f(inputs["z"])
    W0, W1, W2, W3 = f(inputs["W0"]), f(inputs["W1"]), f(inputs["W2"]), f(inputs["W3"])
    inv = np.exp(-f(inputs["m_params"])[:, 0])
    invm64 = np.repeat(inv, 2)
    jnv = np.empty(32, np.float32)
    jnv[0] = inv[0]
    jnv[1:] = inv[:-1] + inv[1:]
    eco = (-4.0 * inv).astype(np.float32)
    eco[0] = 4.0 * inv[0]
    eco[31] = 0.0
    row = np.concatenate([np.tile(invm64, 4), np.tile(4.0 * jnv, 4), np.tile(eco, 4)])
    cc = np.ascontiguousarray(np.broadcast_to(row.astype(np.float32), (128, 512)))
    shared = {
        "w0": W0,
        "w1": W1,
        "w2": W2,
        "w1t": np.ascontiguousarray(W1.T),
        "w2t": np.ascontiguousarray(W2.T),
        "w0t": np.ascontiguousarray(W0.T),
        "bias0": f(inputs["b0"]),
        "bias1": f(inputs["b1"]),
        "bias2": f(inputs["b2"]),
        "w3": np.ascontiguousarray(W3[:, 0]),
        "cc": cc,
    }
    return [
        {**shared, "z": np.ascontiguousarray(z[i * BSL : (i + 1) * BSL])}
        for i in range(NCORES)
    ]


TRACE = False       # set by dev harnesses to capture an NTFF profile
TMPDIR = None       # set by dev harnesses to keep the trace artifacts
LAST_RESULT = None  # BassKernelResults of the most recent run


def kernel(**inputs) -> np.ndarray:
    global LAST_RESULT
    from concourse.bass_utils import run_bass_kernel_spmd

    nc = build_program()
    in_maps = host_inputs(inputs)
    res = run_bass_kernel_spmd(
        nc, in_maps, list(range(NCORES)), trace=TRACE, tmpdir=TMPDIR
    )
    LAST_RESULT = res
    return np.concatenate([res.results[i]["out"] for i in range(NCORES)], axis=0)


# revision 6
# speedup vs baseline: 1.5147x; 1.5147x over previous
"""Trainium2 Bass kernel for the constrained Hamiltonian NN (CHNN) vector field.

Math: the reference solves, per batch sample,
    out = JdH - J DPhi X,   A X = DPhi^T JdH,   A = DPhi^T J DPhi  (64x64)
For chain constraints DPhi = [[Dr, Ddot], [0, Minv Dr]] and
    A = [[0, G], [-G, K]],  G = Dr^T Minv Dr  (32x32 tridiagonal SPD),
    K = Ddot^T Minv Dr - Dr^T Minv Ddot      (tridiagonal antisymmetric)
so X = [x0; x1] with x1 = G^{-1} b0 and x0 = G^{-1}(K x1 - b1), where
    b0 = Dr^T v,  b1 = Ddot^T v - Dr^T Minv g,  v = Minv p,  g = dV/dr.
Output: out_r = v - Minv Dr x1 ; out_p = -g + Dr x0 + Ddot x1.

The MLP grad g runs feature-major on the tensor engine in fp32r (batch on
the free axis); the tridiagonal solves run batch-major on the vector engine
via parallel cyclic reduction (PCR, 5 levels), with per-level coefficients
kept so the second solve is a cheap rhs-only replay.  softplus/sigmoid are
computed with Exp/Ln only so a single activation table serves all ACT ops.

Sharding: pure data-parallel over the batch axis across 8 cores.
"""

import numpy as np

N, D = 32, 2
ZD = 128            # state dim
BS = 4096           # full batch
NCORES = 8
BSL = BS // NCORES  # 512 per core
C4 = BSL // 128     # 4 batch chunks of 128 partitions
AC = 16             # PCR active-window column offset (pad width = max shift)

# packed fp32r weight block column offsets (per partition p):
#   W0[p, :] (256) | W1 2x256 | W2 2x256 | W1T 2x256 | W2T 2x256 | W0T 2x64
OW0, OW1, OW2, OW1T, OW2T, OW0T, WTOT = 0, 256, 768, 1280, 1792, 2304, 2432
# packed fp32 block: b0(2) b1(2) b2(2) w3(2) invm(4x64) jnv4(4x32) fco(4x32) eco(4x32)
OB0, OB1, OB2, OW3, OIV, OJN, OFC, OEC, CTOT = 0, 2, 4, 6, 8, 264, 392, 520, 648


def build_program(debug=False):
    """Build + compile the single-core SPMD Bass/Tile program."""
    from contextlib import ExitStack

    import concourse.bass as bass
    import concourse.mybir as mybir
    import concourse.tile as tile
    from concourse import bacc
    from concourse.hw_specs import get_activation_tables
    from concourse.masks import make_identity
    import bass_rust as _bass_rust

    f32 = mybir.dt.float32
    f32r = mybir.dt.float32r
    AF = mybir.ActivationFunctionType
    OP = mybir.AluOpType

    class PinnedActBacc(bacc.Bacc):
        # Keep every ACT op on one table (Exp+Ln live together in
        # natural_log_exp_and_others); emptying the others preserves the
        # act_func_set_id indexing while forcing a single table load.
        def insert_act_table_loads(self):
            has_activation = any(
                isinstance(i, mybir.InstActivation)
                for b in self.main_func.blocks
                for i in b.instructions
            )
            if not has_activation:
                return
            tables = [
                (name, funcs if name == "natural_log_exp_and_others" else set())
                for name, funcs in get_activation_tables(self.m.arch).items()
            ]
            _bass_rust.insert_act_table_loads(self, tables)

    nc = PinnedActBacc(
        "TRN2",
        target_bir_lowering=False,
        debug=debug,
        enable_asserts=True,
        num_devices=NCORES,
    )

    z = nc.dram_tensor("z", [BSL, ZD], f32, kind="ExternalInput")
    wpk = nc.dram_tensor("wpk", [128, WTOT], f32r, kind="ExternalInput")
    cpk = nc.dram_tensor("cpk", [128, CTOT], f32, kind="ExternalInput")
    out = nc.dram_tensor("out", [BSL, ZD], f32, kind="ExternalOutput")

    with tile.TileContext(nc) as tc:
        with ExitStack() as ctx:
            const = ctx.enter_context(tc.tile_pool(name="const", bufs=1))
            main = ctx.enter_context(tc.tile_pool(name="main", bufs=1))
            scr = ctx.enter_context(tc.tile_pool(name="scr", bufs=2))
            psmm = ctx.enter_context(tc.tile_pool(name="psmm", bufs=4, space="PSUM"))
            pstr = ctx.enter_context(tc.tile_pool(name="pstr", bufs=2, space="PSUM"))

            # ---- constants: two packed DMAs ----
            wsb = const.tile([128, WTOT], f32r)
            nc.sync.dma_start(out=wsb, in_=wpk.ap())
            csb = const.tile([128, CTOT], f32)
            nc.sync.dma_start(out=csb, in_=cpk.ap())

            def wview(off, k, n):
                return wsb[:, off : off + k * n].rearrange("p (k n) -> p k n", k=k)

            w0sb = wsb[0:64, OW0 : OW0 + 256]
            w1sb = wview(OW1, 2, 256)
            w2sb = wview(OW2, 2, 256)
            w1tsb = wview(OW1T, 2, 256)
            w2tsb = wview(OW2T, 2, 256)
            w0tsb = wview(OW0T, 2, 64)
            b0sb = csb[:, OB0 : OB0 + 2]
            b1sb = csb[:, OB1 : OB1 + 2]
            b2sb = csb[:, OB2 : OB2 + 2]
            w3sb = csb[:, OW3 : OW3 + 2]
            invmP = csb[:, OIV : OIV + 256].rearrange("p (c f) -> p c f", f=64)
            jnvP = csb[:, OJN : OJN + 128].rearrange("p (c f) -> p c f", f=32)
            fcoP = csb[:, OFC : OFC + 128].rearrange("p (c f) -> p c f", f=32)
            ecoP = csb[:, OEC : OEC + 128].rearrange("p (c f) -> p c f", f=32)
            ident = const.tile([128, 128], f32)
            make_identity(nc, ident)

            # ---- load z batch-major: [p, chunk, feature] ----
            zt = main.tile([128, C4, ZD], f32)
            nc.gpsimd.dma_start(out=zt, in_=z.ap().rearrange("(c p) f -> p c f", p=128))

            # ================= MLP forward start (PE/ACT heavy) ============
            rT = main.tile([64, BSL], f32r)
            for c in range(C4):
                pt = pstr.tile([64, 128], f32, tag="ptr", name="pt")
                nc.tensor.transpose(pt, zt[:, c, 0:64], ident)
                nc.scalar.copy(rT[:, c * 128 : (c + 1) * 128], pt)

            # softplus h = Ln(Exp(x)+1); sigmoid s = Exp(x - h); single table
            def act_pair(ps, bsl, m, h_dst, s_dst):
                tE = scr.tile([128, BSL], f32, tag="tE", name="tE")
                nc.scalar.activation(tE, ps, AF.Exp, bias=bsl[:, m : m + 1])
                nc.scalar.activation(h_dst, tE, AF.Ln, bias=1.0)
                dq = scr.tile([128, BSL], f32, tag="dq", name="dq")
                nc.vector.scalar_tensor_tensor(
                    dq, ps, bsl[:, m : m + 1], h_dst, op0=OP.add, op1=OP.subtract
                )
                nc.scalar.activation(s_dst, dq, AF.Exp)

            h0 = main.tile([128, 2, BSL], f32r)
            s0 = main.tile([128, 2, BSL], f32)
            for m in range(2):
                ps = psmm.tile([128, BSL], f32, tag="mm", name="ps0")
                nc.tensor.matmul(
                    ps, w0sb[:, m * 128 : (m + 1) * 128], rT, start=True, stop=True
                )
                act_pair(ps, b0sb, m, h0[:, m, :], s0[:, m, :])

            # ---- chain quantities (batch-major, vector engine) ----
            vt = main.tile([128, C4, 64], f32)       # v = Minv p
            nc.vector.tensor_mul(vt, zt[:, :, 64:128], invmP)
            ut = main.tile([128, C4, 64], f32)       # u~ = diff(r) (u = 2 u~)
            nc.vector.tensor_sub(ut[:, :, 2:64], zt[:, :, 0:62], zt[:, :, 2:64])
            nc.gpsimd.tensor_copy(ut[:, :, 0:2], zt[:, :, 0:2])
            wt = main.tile([128, C4, 64], f32)       # w~ = diff(v)
            nc.vector.tensor_sub(wt[:, :, 2:64], vt[:, :, 0:62], vt[:, :, 2:64])
            nc.gpsimd.tensor_copy(wt[:, :, 0:2], vt[:, :, 0:2])

            def prodpair(dst, xa, ya, npair):
                # dst[:, :, 0:npair] = pairwise-D sum of xa*ya
                pr = scr.tile([128, C4, 64], f32, tag="prod", name="pr")
                nc.vector.tensor_mul(pr[:, :, 0 : 2 * npair], xa, ya)
                pe = pr.rearrange("p c (i d) -> p c i d", d=2)
                nc.vector.tensor_add(
                    dst[:, :, 0:npair], pe[:, :, 0:npair, 0], pe[:, :, 0:npair, 1]
                )

            dcc = main.tile([128, C4, 32], f32)
            prodpair(dcc, ut, ut, 32)
            puw = main.tile([128, C4, 32], f32)
            prodpair(puw, ut, wt, 32)
            pww = main.tile([128, C4, 32], f32)
            prodpair(pww, wt, wt, 32)
            cuu = main.tile([128, C4, 32], f32)
            prodpair(cuu, ut[:, :, 0:62], ut[:, :, 2:64], 31)
            cwu = main.tile([128, C4, 32], f32)
            prodpair(cwu, wt[:, :, 0:62], ut[:, :, 2:64], 31)
            cuw = main.tile([128, C4, 32], f32)
            prodpair(cuw, ut[:, :, 0:62], wt[:, :, 2:64], 31)

            # ---- PCR assembly (padded tiles, active cols AC..AC+32).
            # System: a_i x_i = b_i + f_{i-s} x_{i-s} + f_i x_{i+s}, f = -e.
            apcr = main.tile([128, C4, 64], f32)
            nc.gpsimd.memset(apcr, 1.0)
            ft = [main.tile([128, C4, 64], f32, tag=f"ft{i}", name=f"ft{i}") for i in range(2)]
            bt = [main.tile([128, C4, 64], f32, tag=f"bt{i}", name=f"bt{i}") for i in range(2)]
            for tl_ in (*ft, *bt):
                nc.gpsimd.memset(tl_, 0.0)
            kkp = main.tile([128, C4, 64], f32)
            nc.gpsimd.memset(kkp, 0.0)
            x1p = main.tile([128, C4, 64], f32)
            nc.gpsimd.memset(x1p, 0.0)
            x0p = main.tile([128, C4, 64], f32)
            nc.gpsimd.memset(x0p, 0.0)

            def A_(t, off=0, w=32):
                return t[:, :, AC + off : AC + off + w]

            nc.vector.tensor_mul(A_(apcr), jnvP, dcc)
            nc.vector.tensor_mul(A_(ft[0], 0, 31), fcoP[:, :, 0:31], cuu[:, :, 0:31])
            nc.vector.tensor_scalar_mul(A_(bt[0]), puw, 2.0)
            cdt = scr.tile([128, C4, 32], f32, tag="cdt", name="cdt")
            nc.vector.tensor_sub(cdt[:, :, 0:31], cwu[:, :, 0:31], cuw[:, :, 0:31])
            nc.vector.tensor_mul(A_(kkp, 0, 31), ecoP[:, :, 0:31], cdt[:, :, 0:31])

            # ---- PCR level helper (factorize+solve1); tl/ql kept for replay
            rt_ = main.tile([128, C4, 64], f32)
            rs_ = main.tile([128, C4, 64], f32)   # recip scratch
            tl = [main.tile([128, C4, 64], f32, tag=f"tl{i}", name=f"tl{i}") for i in range(5)]
            ql = [main.tile([128, C4, 32], f32, tag=f"ql{i}", name=f"ql{i}") for i in range(5)]
            shifts = [1, 2, 4, 8, 16]

            def pcr_level(lev, cur):
                s = shifts[lev]
                fC, fN = ft[cur], ft[1 - cur]
                bC, bN = bt[cur], bt[1 - cur]
                nc.vector.reciprocal_approx_accurate(rt_, apcr, rs_)
                nc.vector.tensor_mul(tl[lev], fC, rt_)
                nc.vector.tensor_mul(ql[lev], A_(fC), A_(rt_, s))
                u2 = scr.tile([128, C4, 64], f32, tag="u2", name="u2")
                nc.vector.tensor_mul(u2, tl[lev], fC)
                w2 = scr.tile([128, C4, 32], f32, tag="w2", name="w2")
                nc.vector.tensor_mul(w2, ql[lev], A_(fC))
                nc.vector.tensor_sub(A_(apcr), A_(apcr), A_(u2, -s))
                nc.vector.tensor_sub(A_(apcr), A_(apcr), w2)
                tb = scr.tile([128, C4, 64], f32, tag="tb", name="tb")
                nc.vector.tensor_mul(tb, tl[lev], bC)
                m2 = scr.tile([128, C4, 32], f32, tag="m2", name="m2")
                nc.vector.tensor_mul(m2, ql[lev], A_(bC, s))
                nc.vector.tensor_add(A_(bN), A_(bC), A_(tb, -s))
                nc.vector.tensor_add(A_(bN), A_(bN), m2)
                nc.vector.tensor_mul(A_(fN), ql[lev], A_(fC, s))

            pcr_level(0, 0)
            pcr_level(1, 1)

            # ================= MLP layer 1 =================
            h1 = main.tile([128, 2, BSL], f32r)
            s1 = main.tile([128, 2, BSL], f32)
            for m in range(2):
                ps = psmm.tile([128, BSL], f32, tag="mm", name="ps1")
                for k in range(2):
                    nc.tensor.matmul(
                        ps,
                        w1sb[:, k, m * 128 : (m + 1) * 128],
                        h0[:, k, :],
                        start=(k == 0),
                        stop=(k == 1),
                    )
                act_pair(ps, b1sb, m, h1[:, m, :], s1[:, m, :])

            pcr_level(2, 0)
            pcr_level(3, 1)

            # ================= MLP layer 2 + backward =================
            dp2 = main.tile([128, 2, BSL], f32r)
            for m in range(2):
                ps = psmm.tile([128, BSL], f32, tag="mm", name="ps2")
                for k in range(2):
                    nc.tensor.matmul(
                        ps,
                        w2sb[:, k, m * 128 : (m + 1) * 128],
                        h1[:, k, :],
                        start=(k == 0),
                        stop=(k == 1),
                    )
                # layer 2: only the sigmoid is needed (h2 feeds V, unused)
                h2m = scr.tile([128, BSL], f32, tag="h2m", name="h2m")
                s2m = scr.tile([128, BSL], f32, tag="s2m", name="s2m")
                act_pair(ps, b2sb, m, h2m, s2m)
                nc.vector.tensor_scalar_mul(dp2[:, m, :], s2m, w3sb[:, m : m + 1])

            pcr_level(4, 0)
            nc.vector.reciprocal_approx_accurate(rt_, apcr, rs_)
            nc.vector.tensor_mul(A_(x1p), A_(bt[1]), A_(rt_))

            dp1 = main.tile([128, 2, BSL], f32r)
            for m in range(2):
                ps = psmm.tile([128, BSL], f32, tag="mm", name="ps3")
                for k in range(2):
                    nc.tensor.matmul(
                        ps,
                        w2tsb[:, k, m * 128 : (m + 1) * 128],
                        dp2[:, k, :],
                        start=(k == 0),
                        stop=(k == 1),
                    )
                nc.vector.tensor_mul(dp1[:, m, :], ps, s1[:, m, :])
            dp0 = main.tile([128, 2, BSL], f32r)
            for m in range(2):
                ps = psmm.tile([128, BSL], f32, tag="mm", name="ps4")
                for k in range(2):
                    nc.tensor.matmul(
                        ps,
                        w1tsb[:, k, m * 128 : (m + 1) * 128],
                        dp1[:, k, :],
                        start=(k == 0),
                        stop=(k == 1),
                    )
                nc.vector.tensor_mul(dp0[:, m, :], ps, s0[:, m, :])
            gps = psmm.tile([64, BSL], f32, tag="mm", name="gps")
            for k in range(2):
                nc.tensor.matmul(
                    gps, w0tsb[:, k, :], dp0[:, k, :], start=(k == 0), stop=(k == 1)
                )
            gsb = main.tile([64, BSL], f32)
            nc.scalar.copy(gsb, gps)
            gbm = main.tile([128, C4, 64], f32)   # g batch-major
            for c in range(C4):
                pt2 = pstr.tile([128, 64], f32, tag="ptg", name="pt2")
                nc.tensor.transpose(
                    pt2, gsb[:, c * 128 : (c + 1) * 128], ident[0:64, 0:64]
                )
                nc.scalar.copy(gbm[:, c, :], pt2)

            # ---- b1 pieces that need g ----
            gt = main.tile([128, C4, 64], f32)    # Minv g
            nc.vector.tensor_mul(gt, gbm, invmP)
            gd = main.tile([128, C4, 64], f32)    # chain-diff of Minv g
            nc.vector.tensor_sub(gd[:, :, 2:64], gt[:, :, 0:62], gt[:, :, 2:64])
            nc.gpsimd.tensor_copy(gd[:, :, 0:2], gt[:, :, 0:2])
            pugd = main.tile([128, C4, 32], f32)
            prodpair(pugd, ut, gd, 32)
            b1t = main.tile([128, C4, 32], f32)   # (b1 = 2*b1t)
            nc.vector.tensor_sub(b1t, pww, pugd)

            # ---- rhs2 = K x1 - b1; solve G x0 = rhs2 by replaying PCR ----
            OP_ = OP
            t1 = scr.tile([128, C4, 32], f32, tag="t1", name="t1")
            nc.vector.tensor_mul(t1, A_(kkp), A_(x1p, 1))
            t2k = scr.tile([128, C4, 32], f32, tag="t2k", name="t2k")
            nc.vector.tensor_mul(t2k, A_(kkp, -1), A_(x1p, -1))
            ttk = scr.tile([128, C4, 32], f32, tag="ttk", name="ttk")
            nc.vector.tensor_sub(ttk, t1, t2k)
            nc.vector.scalar_tensor_tensor(
                A_(bt[0]), b1t, -2.0, ttk, op0=OP_.mult, op1=OP_.add
            )
            cur = 0
            for lev, s in enumerate(shifts):
                bC, bN = bt[cur], bt[1 - cur]
                tb = scr.tile([128, C4, 64], f32, tag="tb", name="tb")
                nc.vector.tensor_mul(tb, tl[lev], bC)
                m2 = scr.tile([128, C4, 32], f32, tag="m2", name="m2")
                nc.vector.tensor_mul(m2, ql[lev], A_(bC, s))
                nc.vector.tensor_add(A_(bN), A_(bC), A_(tb, -s))
                nc.vector.tensor_add(A_(bN), A_(bN), m2)
                cur = 1 - cur
            nc.vector.tensor_mul(A_(x0p), A_(bt[1]), A_(rt_))

            # ---- outputs ----
            x1e = main.tile([128, C4, 64], f32)
            x0e = main.tile([128, C4, 64], f32)
            for xe, xp in ((x1e, x1p), (x0e, x0p)):
                xv = xe.rearrange("p c (i d) -> p c i d", d=2)
                nc.vector.tensor_scalar_mul(xv[:, :, :, 0], A_(xp), 2.0)
                nc.vector.tensor_scalar_mul(xv[:, :, :, 1], A_(xp), 2.0)
            A1 = main.tile([128, C4, 64], f32)    # x1 * u  (per feature)
            nc.vector.tensor_mul(A1, x1e, ut)
            Bt1 = main.tile([128, C4, 64], f32)   # x0*u + x1*w
            nc.vector.tensor_mul(Bt1, x0e, ut)
            bs_ = scr.tile([128, C4, 64], f32, tag="bs_", name="bs_")
            nc.vector.tensor_mul(bs_, x1e, wt)
            nc.vector.tensor_add(Bt1, Bt1, bs_)

            outt = main.tile([128, C4, ZD], f32)

            def drx(dst, src):
                # dst = Dr-combine(src): node0: s0+s1; mid: s_{i+1}-s_i; last: -s
                nc.vector.tensor_add(dst[:, :, 0:2], src[:, :, 2:4], src[:, :, 0:2])
                nc.vector.tensor_sub(dst[:, :, 2:62], src[:, :, 4:64], src[:, :, 2:62])
                nc.vector.tensor_scalar_mul(dst[:, :, 62:64], src[:, :, 62:64], -1.0)

            drA = main.tile([128, C4, 64], f32)
            drx(drA, A1)
            sD = scr.tile([128, C4, 64], f32, tag="sD", name="sD")
            nc.vector.tensor_mul(sD, drA, invmP)
            nc.vector.tensor_sub(outt[:, :, 0:64], vt, sD)
            drB = main.tile([128, C4, 64], f32)
            drx(drB, Bt1)
            nc.vector.tensor_sub(outt[:, :, 64:128], drB, gbm)

            nc.sync.dma_start(
                out=out.ap().rearrange("(c p) f -> p c f", p=128), in_=outt
            )

    nc.compile()
    return nc


def host_inputs(inputs):
    """Host-side prep: per-core input maps (weights replicated, z sharded)."""
    f = lambda x: np.ascontiguousarray(np.asarray(x, np.float32))
    z = f(inputs["z"])
    W0, W1, W2, W3 = f(inputs["W0"]), f(inputs["W1"]), f(inputs["W2"]), f(inputs["W3"])

    wpk = np.zeros((128, WTOT), np.float32)
    wpk[0:64, OW0 : OW0 + 256] = W0
    for k in range(2):
        sl = slice(k * 128, (k + 1) * 128)
        wpk[:, OW1 + 256 * k : OW1 + 256 * (k + 1)] = W1[sl]
        wpk[:, OW2 + 256 * k : OW2 + 256 * (k + 1)] = W2[sl]
        wpk[:, OW1T + 256 * k : OW1T + 256 * (k + 1)] = W1.T[sl]
        wpk[:, OW2T + 256 * k : OW2T + 256 * (k + 1)] = W2.T[sl]
        wpk[:, OW0T + 64 * k : OW0T + 64 * (k + 1)] = W0.T[sl]

    inv = np.exp(-f(inputs["m_params"])[:, 0])
    invm64 = np.repeat(inv, 2)
    jnv = np.empty(32, np.float32)
    jnv[0] = inv[0]
    jnv[1:] = inv[:-1] + inv[1:]
    eco = (-4.0 * inv).astype(np.float32)   # sigma_c * 4 * inv_c, c>=1
    eco[0] = 4.0 * inv[0]
    eco[31] = 0.0
    row = np.zeros(CTOT, np.float32)
    row[OIV : OIV + 256] = np.tile(invm64, 4)
    row[OJN : OJN + 128] = np.tile(4.0 * jnv, 4)
    row[OFC : OFC + 128] = np.tile(-eco, 4)   # f = -e
    row[OEC : OEC + 128] = np.tile(eco, 4)
    cpk = np.broadcast_to(row, (128, CTOT)).copy()
    for off, b in ((OB0, inputs["b0"]), (OB1, inputs["b1"]), (OB2, inputs["b2"])):
        cpk[:, off : off + 2] = f(b).reshape(2, 128).T
    cpk[:, OW3 : OW3 + 2] = W3[:, 0].reshape(2, 128).T
    cpk = np.ascontiguousarray(cpk)

    shared = {"wpk": np.ascontiguousarray(wpk), "cpk": cpk}
    return [
        {**shared, "z": np.ascontiguousarray(z[i * BSL : (i + 1) * BSL])}
        for i in range(NCORES)
    ]


TRACE = False       # set by dev harnesses to capture an NTFF profile
TMPDIR = None       # set by dev harnesses to keep the trace artifacts
LAST_RESULT = None  # BassKernelResults of the most recent run


def kernel(**inputs) -> np.ndarray:
    global LAST_RESULT
    from concourse.bass_utils import run_bass_kernel_spmd

    nc = build_program()
    in_maps = host_inputs(inputs)
    res = run_bass_kernel_spmd(
        nc, in_maps, list(range(NCORES)), trace=TRACE, tmpdir=TMPDIR
    )
    LAST_RESULT = res
    return np.concatenate([res.results[i]["out"] for i in range(NCORES)], axis=0)


# revision 10
# speedup vs baseline: 1.5639x; 1.0325x over previous
"""Trainium2 Bass kernel for the constrained Hamiltonian NN (CHNN) vector field.

Math: the reference solves, per batch sample,
    out = JdH - J DPhi X,   A X = DPhi^T JdH,   A = DPhi^T J DPhi  (64x64)
For chain constraints DPhi = [[Dr, Ddot], [0, Minv Dr]] and
    A = [[0, G], [-G, K]],  G = Dr^T Minv Dr  (32x32 tridiagonal SPD),
    K = Ddot^T Minv Dr - Dr^T Minv Ddot      (tridiagonal antisymmetric)
so X = [x0; x1] with x1 = G^{-1} b0 and x0 = G^{-1}(K x1 - b1), where
    b0 = Dr^T v,  b1 = Ddot^T v - Dr^T Minv g,  v = Minv p,  g = dV/dr.
Output: out_r = v - Minv Dr x1 ; out_p = -g + Dr x0 + Ddot x1.

The MLP grad g runs feature-major on the tensor engine in fp32r (batch on
the free axis); the tridiagonal solves run batch-major on the vector engine
via parallel cyclic reduction (PCR, 5 levels), with per-level coefficients
kept so the second solve is a cheap rhs-only replay.  softplus/sigmoid are
computed with Exp/Ln only so a single activation table serves all ACT ops.

Sharding: pure data-parallel over the batch axis across 8 cores.
"""

import numpy as np

N, D = 32, 2
ZD = 128            # state dim
BS = 4096           # full batch
NCORES = 8
BSL = BS // NCORES  # 512 per core
C4 = BSL // 128     # 4 batch chunks of 128 partitions
AC = 16             # PCR active-window column offset (pad width = max shift)

# packed fp32r weight block column offsets (per partition p):
#   W0[p, :] (256) | W1 2x256 | W2 2x256 | W1T 2x256 | W2T 2x256 | W0T 2x64
OW0, OW1, OW2, OW1T, OW2T, OW0T, WTOT = 0, 256, 768, 1280, 1792, 2304, 2432
# packed fp32 block: b0(2) b1(2) b2(2) w3(2) invm(4x64) jnv4(4x32) fco(4x32) eco(4x32)
OB0, OB1, OB2, OW3, OIV, OJN, OFC, OEC, CTOT = 0, 2, 4, 6, 8, 264, 392, 520, 648


def build_program(debug=False):
    """Build + compile the single-core SPMD Bass/Tile program."""
    from contextlib import ExitStack

    import concourse.bass as bass
    import concourse.mybir as mybir
    import concourse.tile as tile
    from concourse import bacc
    from concourse.hw_specs import get_activation_tables
    from concourse.masks import make_identity
    import bass_rust as _bass_rust

    f32 = mybir.dt.float32
    f32r = mybir.dt.float32r
    AF = mybir.ActivationFunctionType
    OP = mybir.AluOpType

    class PinnedActBacc(bacc.Bacc):
        # Keep every ACT op on one table (Exp+Ln live together in
        # natural_log_exp_and_others); emptying the others preserves the
        # act_func_set_id indexing while forcing a single table load.
        def insert_act_table_loads(self):
            has_activation = any(
                isinstance(i, mybir.InstActivation)
                for b in self.main_func.blocks
                for i in b.instructions
            )
            if not has_activation:
                return
            tables = [
                (name, funcs if name == "natural_log_exp_and_others" else set())
                for name, funcs in get_activation_tables(self.m.arch).items()
            ]
            _bass_rust.insert_act_table_loads(self, tables)

    nc = PinnedActBacc(
        "TRN2",
        target_bir_lowering=False,
        debug=debug,
        enable_asserts=True,
        num_devices=NCORES,
    )

    z = nc.dram_tensor("z", [BSL, ZD], f32, kind="ExternalInput")
    wpk = nc.dram_tensor("wpk", [128, WTOT], f32r, kind="ExternalInput")
    cpk = nc.dram_tensor("cpk", [128, CTOT], f32, kind="ExternalInput")
    out = nc.dram_tensor("out", [BSL, ZD], f32, kind="ExternalOutput")

    with tile.TileContext(nc) as tc:
        with ExitStack() as ctx:
            const = ctx.enter_context(tc.tile_pool(name="const", bufs=1))
            main = ctx.enter_context(tc.tile_pool(name="main", bufs=1))
            scr = ctx.enter_context(tc.tile_pool(name="scr", bufs=2))
            psmm = ctx.enter_context(tc.tile_pool(name="psmm", bufs=4, space="PSUM"))
            pstr = ctx.enter_context(tc.tile_pool(name="pstr", bufs=2, space="PSUM"))

            # ---- constants: two packed DMAs ----
            wsb = const.tile([128, WTOT], f32r)
            nc.sync.dma_start(out=wsb, in_=wpk.ap())
            csb = const.tile([128, CTOT], f32)
            nc.sync.dma_start(out=csb, in_=cpk.ap())

            def wview(off, k, n):
                return wsb[:, off : off + k * n].rearrange("p (k n) -> p k n", k=k)

            w0sb = wsb[0:64, OW0 : OW0 + 256]
            w1sb = wview(OW1, 2, 256)
            w2sb = wview(OW2, 2, 256)
            w1tsb = wview(OW1T, 2, 256)
            w2tsb = wview(OW2T, 2, 256)
            w0tsb = wview(OW0T, 2, 64)
            b0sb = csb[:, OB0 : OB0 + 2]
            b1sb = csb[:, OB1 : OB1 + 2]
            b2sb = csb[:, OB2 : OB2 + 2]
            w3sb = csb[:, OW3 : OW3 + 2]
            invmP = csb[:, OIV : OIV + 256].rearrange("p (c f) -> p c f", f=64)
            jnvP = csb[:, OJN : OJN + 128].rearrange("p (c f) -> p c f", f=32)
            fcoP = csb[:, OFC : OFC + 128].rearrange("p (c f) -> p c f", f=32)
            ecoP = csb[:, OEC : OEC + 128].rearrange("p (c f) -> p c f", f=32)
            ident = const.tile([128, 128], f32)
            make_identity(nc, ident)

            # ---- load z batch-major: [p, chunk, feature] ----
            zt = main.tile([128, C4, ZD], f32)
            nc.gpsimd.dma_start(out=zt, in_=z.ap().rearrange("(c p) f -> p c f", p=128))

            # ================= MLP forward start (PE/ACT heavy) ============
            rT = main.tile([64, BSL], f32r)
            for c in range(C4):
                pt = pstr.tile([64, 128], f32, tag="ptr", name="pt")
                nc.tensor.transpose(pt, zt[:, c, 0:64], ident)
                nc.scalar.copy(rT[:, c * 128 : (c + 1) * 128], pt)

            # softplus h = Ln(Exp(x)+1); then e = Exp(-h) so that
            # sigmoid = 1 - e.  All ACT ops stay in one table (Exp/Ln).
            def act_pair(ps, bsl, m, h_dst, e_dst):
                tE = scr.tile([128, BSL], f32, tag="tE", name="tE")
                nc.scalar.activation(tE, ps, AF.Exp, bias=bsl[:, m : m + 1])
                nc.scalar.activation(h_dst, tE, AF.Ln, bias=1.0)
                nc.scalar.activation(e_dst, h_dst, AF.Exp, scale=-1.0)

            h0 = main.tile([128, 2, BSL], f32r)
            e0 = main.tile([128, 2, BSL], f32)
            for m in range(2):
                ps = psmm.tile([128, BSL], f32, tag="mm", name="ps0")
                nc.tensor.matmul(
                    ps, w0sb[:, m * 128 : (m + 1) * 128], rT, start=True, stop=True
                )
                act_pair(ps, b0sb, m, h0[:, m, :], e0[:, m, :])

            # ---- chain quantities (batch-major, vector engine) ----
            vt = main.tile([128, C4, 64], f32)       # v = Minv p
            nc.vector.tensor_mul(vt, zt[:, :, 64:128], invmP)
            ut = main.tile([128, C4, 64], f32)       # u~ = diff(r) (u = 2 u~)
            nc.vector.tensor_sub(ut[:, :, 2:64], zt[:, :, 0:62], zt[:, :, 2:64])
            nc.gpsimd.tensor_copy(ut[:, :, 0:2], zt[:, :, 0:2])
            wt = main.tile([128, C4, 64], f32)       # w~ = diff(v)
            nc.vector.tensor_sub(wt[:, :, 2:64], vt[:, :, 0:62], vt[:, :, 2:64])
            nc.gpsimd.tensor_copy(wt[:, :, 0:2], vt[:, :, 0:2])

            def prodpair(dst, xa, ya, npair, mul_eng=None):
                # dst[:, :, 0:npair] = pairwise-D sum of xa*ya
                pr = scr.tile([128, C4, 64], f32, tag="prod", name="pr")
                (mul_eng or nc.vector).tensor_mul(pr[:, :, 0 : 2 * npair], xa, ya)
                pe = pr.rearrange("p c (i d) -> p c i d", d=2)
                nc.vector.tensor_add(
                    dst[:, :, 0:npair], pe[:, :, 0:npair, 0], pe[:, :, 0:npair, 1]
                )

            dcc = main.tile([128, C4, 32], f32)
            prodpair(dcc, ut, ut, 32)
            puw = main.tile([128, C4, 32], f32)
            prodpair(puw, ut, wt, 32)
            pww = main.tile([128, C4, 32], f32)
            prodpair(pww, wt, wt, 32, nc.gpsimd)
            cuu = main.tile([128, C4, 32], f32)
            prodpair(cuu, ut[:, :, 0:62], ut[:, :, 2:64], 31)
            cwu = main.tile([128, C4, 32], f32)
            prodpair(cwu, wt[:, :, 0:62], ut[:, :, 2:64], 31, nc.gpsimd)
            cuw = main.tile([128, C4, 32], f32)
            prodpair(cuw, ut[:, :, 0:62], wt[:, :, 2:64], 31, nc.gpsimd)

            # ---- PCR assembly (padded tiles, active cols AC..AC+32).
            # System: a_i x_i = b_i + f_{i-s} x_{i-s} + f_i x_{i+s}, f = -e.
            apcr = main.tile([128, C4, 64], f32)
            nc.gpsimd.memset(apcr, 1.0)
            ft = [main.tile([128, C4, 64], f32, tag=f"ft{i}", name=f"ft{i}") for i in range(2)]
            bt = [main.tile([128, C4, 64], f32, tag=f"bt{i}", name=f"bt{i}") for i in range(2)]
            for tl_ in (*ft, *bt):
                nc.gpsimd.memset(tl_, 0.0)
            kkp = main.tile([128, C4, 64], f32)
            nc.gpsimd.memset(kkp, 0.0)
            x1p = main.tile([128, C4, 64], f32)
            nc.gpsimd.memset(x1p, 0.0)
            x0p = main.tile([128, C4, 64], f32)
            nc.gpsimd.memset(x0p, 0.0)

            def A_(t, off=0, w=32):
                return t[:, :, AC + off : AC + off + w]

            nc.vector.tensor_mul(A_(apcr), jnvP, dcc)
            nc.vector.tensor_mul(A_(ft[0], 0, 31), fcoP[:, :, 0:31], cuu[:, :, 0:31])
            nc.scalar.activation(A_(bt[0]), puw, AF.Copy, scale=2.0)
            cdt = scr.tile([128, C4, 32], f32, tag="cdt", name="cdt")
            nc.vector.tensor_sub(cdt[:, :, 0:31], cwu[:, :, 0:31], cuw[:, :, 0:31])
            nc.vector.tensor_mul(A_(kkp, 0, 31), ecoP[:, :, 0:31], cdt[:, :, 0:31])

            # ---- PCR level helper (factorize+solve1); tl/ql kept for replay
            rt_ = main.tile([128, C4, 64], f32)
            rs_ = main.tile([128, C4, 64], f32)   # recip scratch
            tl = [main.tile([128, C4, 64], f32, tag=f"tl{i}", name=f"tl{i}") for i in range(5)]
            ql = [main.tile([128, C4, 32], f32, tag=f"ql{i}", name=f"ql{i}") for i in range(5)]
            shifts = [1, 2, 4, 8, 16]

            def pcr_level(lev, cur):
                # t/q kept for the rhs-only replay; the b-path runs on
                # gpsimd so solve-1 overlaps the MLP without loading DVE.
                s = shifts[lev]
                fC, fN = ft[cur], ft[1 - cur]
                bC, bN = bt[cur], bt[1 - cur]
                nc.vector.reciprocal_approx_accurate(
                    A_(rt_, -s, 32 + 2 * s), A_(apcr, -s, 32 + 2 * s),
                    A_(rs_, -s, 32 + 2 * s),
                )
                nc.vector.tensor_mul(A_(tl[lev], -s, 32 + s), A_(fC, -s, 32 + s),
                                     A_(rt_, -s, 32 + s))
                nc.vector.tensor_mul(ql[lev], A_(fC), A_(rt_, s))
                u2 = scr.tile([128, C4, 64], f32, tag="u2", name="u2")
                nc.vector.tensor_mul(A_(u2, -s), A_(tl[lev], -s), A_(fC, -s))
                w2 = scr.tile([128, C4, 32], f32, tag="w2", name="w2")
                nc.vector.tensor_mul(w2, ql[lev], A_(fC))
                nc.vector.tensor_sub(A_(apcr), A_(apcr), A_(u2, -s))
                nc.vector.tensor_sub(A_(apcr), A_(apcr), w2)
                tb = scr.tile([128, C4, 64], f32, tag="tb", name="tb")
                nc.gpsimd.tensor_mul(A_(tb, -s), A_(tl[lev], -s), A_(bC, -s))
                m2 = scr.tile([128, C4, 32], f32, tag="m2", name="m2")
                nc.gpsimd.tensor_mul(m2, ql[lev], A_(bC, s))
                nc.gpsimd.tensor_add(A_(bN), A_(bC), A_(tb, -s))
                nc.gpsimd.tensor_add(A_(bN), A_(bN), m2)
                nc.vector.tensor_mul(A_(fN), ql[lev], A_(fC, s))

            pcr_level(0, 0)
            pcr_level(1, 1)

            # ================= MLP layer 1 =================
            h1 = main.tile([128, 2, BSL], f32r)
            e1 = main.tile([128, 2, BSL], f32)
            for m in range(2):
                ps = psmm.tile([128, BSL], f32, tag="mm", name="ps1")
                for k in range(2):
                    nc.tensor.matmul(
                        ps,
                        w1sb[:, k, m * 128 : (m + 1) * 128],
                        h0[:, k, :],
                        start=(k == 0),
                        stop=(k == 1),
                    )
                act_pair(ps, b1sb, m, h1[:, m, :], e1[:, m, :])

            pcr_level(2, 0)
            pcr_level(3, 1)

            # ================= MLP layer 2 + backward =================
            dp2 = main.tile([128, 2, BSL], f32r)
            for m in range(2):
                ps = psmm.tile([128, BSL], f32, tag="mm", name="ps2")
                for k in range(2):
                    nc.tensor.matmul(
                        ps,
                        w2sb[:, k, m * 128 : (m + 1) * 128],
                        h1[:, k, :],
                        start=(k == 0),
                        stop=(k == 1),
                    )
                # layer 2: dp2 holds the NEGATED dpre2 = w3*e2 - w3;
                # the whole backward chain and g are carried negated.
                h2m = scr.tile([128, BSL], f32, tag="h2m", name="h2m")
                e2m = scr.tile([128, BSL], f32, tag="e2m", name="e2m")
                act_pair(ps, b2sb, m, h2m, e2m)
                q2 = scr.tile([128, BSL], f32, tag="q2", name="q2")
                nc.vector.tensor_scalar_mul(q2, e2m, w3sb[:, m : m + 1])
                nc.vector.tensor_scalar_sub(dp2[:, m, :], q2, w3sb[:, m : m + 1])

            pcr_level(4, 0)
            nc.vector.reciprocal_approx_accurate(A_(rt_), A_(apcr), A_(rs_))
            nc.vector.tensor_mul(A_(x1p), A_(bt[1]), A_(rt_))

            # ---- x1-dependent outputs (out_r) overlap the MLP backward ----
            outt = main.tile([128, C4, ZD], f32)
            outv = out.ap().rearrange("(c p) f -> p c f", p=128)
            x1e = main.tile([128, C4, 64], f32)
            xv1 = x1e.rearrange("p c (i d) -> p c i d", d=2)
            nc.scalar.activation(xv1[:, :, :, 0], A_(x1p), AF.Copy, scale=2.0)
            nc.scalar.activation(xv1[:, :, :, 1], A_(x1p), AF.Copy, scale=2.0)

            def drx(dst, src):
                # dst = Dr-combine(src): node0: s0+s1; mid: s_{i+1}-s_i; last: -s
                nc.vector.tensor_add(dst[:, :, 0:2], src[:, :, 2:4], src[:, :, 0:2])
                nc.vector.tensor_sub(dst[:, :, 2:62], src[:, :, 4:64], src[:, :, 2:62])
                nc.scalar.activation(
                    dst[:, :, 62:64], src[:, :, 62:64], AF.Copy, scale=-1.0
                )

            A1 = main.tile([128, C4, 64], f32)    # x1 * u  (per feature)
            nc.vector.tensor_mul(A1, x1e, ut)
            drA = main.tile([128, C4, 64], f32)
            drx(drA, A1)
            sD = scr.tile([128, C4, 64], f32, tag="sD", name="sD")
            nc.vector.tensor_mul(sD, drA, invmP)
            nc.vector.tensor_sub(outt[:, :, 0:64], vt, sD)
            nc.sync.dma_start(out=outv[:, :, 0:64], in_=outt[:, :, 0:64])
            bs_ = scr.tile([128, C4, 64], f32, tag="bs_", name="bs_")
            nc.vector.tensor_mul(bs_, x1e, wt)

            dp1 = main.tile([128, 2, BSL], f32r)
            for m in range(2):
                ps = psmm.tile([128, BSL], f32, tag="mm", name="ps3")
                for k in range(2):
                    nc.tensor.matmul(
                        ps,
                        w2tsb[:, k, m * 128 : (m + 1) * 128],
                        dp2[:, k, :],
                        start=(k == 0),
                        stop=(k == 1),
                    )
                qb = scr.tile([128, BSL], f32, tag="qb", name="qb")
                nc.vector.tensor_mul(qb, ps, e1[:, m, :])
                nc.vector.tensor_sub(dp1[:, m, :], ps, qb)
            dp0 = main.tile([128, 2, BSL], f32r)
            for m in range(2):
                ps = psmm.tile([128, BSL], f32, tag="mm", name="ps4")
                for k in range(2):
                    nc.tensor.matmul(
                        ps,
                        w1tsb[:, k, m * 128 : (m + 1) * 128],
                        dp1[:, k, :],
                        start=(k == 0),
                        stop=(k == 1),
                    )
                qb = scr.tile([128, BSL], f32, tag="qb", name="qb")
                nc.vector.tensor_mul(qb, ps, e0[:, m, :])
                nc.vector.tensor_sub(dp0[:, m, :], ps, qb)
            gps = psmm.tile([64, BSL], f32, tag="mm", name="gps")
            for k in range(2):
                nc.tensor.matmul(
                    gps, w0tsb[:, k, :], dp0[:, k, :], start=(k == 0), stop=(k == 1)
                )
            gsb = main.tile([64, BSL], f32)
            nc.scalar.copy(gsb, gps)
            gbm = main.tile([128, C4, 64], f32)   # -g batch-major (negated)
            for c in range(C4):
                pt2 = pstr.tile([128, 64], f32, tag="ptg", name="pt2")
                nc.tensor.transpose(
                    pt2, gsb[:, c * 128 : (c + 1) * 128], ident[0:64, 0:64]
                )
                nc.scalar.copy(gbm[:, c, :], pt2)

            # ---- b1 pieces that need g (gt/gd/pugd all carry -g signs) ----
            gt = main.tile([128, C4, 64], f32)    # -Minv g
            nc.vector.tensor_mul(gt, gbm, invmP)
            gd = main.tile([128, C4, 64], f32)    # -chain-diff of Minv g
            nc.vector.tensor_sub(gd[:, :, 2:64], gt[:, :, 0:62], gt[:, :, 2:64])
            nc.gpsimd.tensor_copy(gd[:, :, 0:2], gt[:, :, 0:2])
            pugd = main.tile([128, C4, 32], f32)  # = -(u~ . Gd)
            prodpair(pugd, ut, gd, 32)
            b1t = main.tile([128, C4, 32], f32)   # (b1 = 2*b1t)
            nc.vector.tensor_add(b1t, pww, pugd)

            # ---- rhs2 = K x1 - b1; solve G x0 = rhs2 by replaying PCR ----
            OP_ = OP
            t1 = scr.tile([128, C4, 32], f32, tag="t1", name="t1")
            nc.vector.tensor_mul(t1, A_(kkp), A_(x1p, 1))
            t2k = scr.tile([128, C4, 32], f32, tag="t2k", name="t2k")
            nc.vector.tensor_mul(t2k, A_(kkp, -1), A_(x1p, -1))
            ttk = scr.tile([128, C4, 32], f32, tag="ttk", name="ttk")
            nc.vector.tensor_sub(ttk, t1, t2k)
            nc.vector.scalar_tensor_tensor(
                A_(bt[0]), b1t, -2.0, ttk, op0=OP_.mult, op1=OP_.add
            )
            cur = 0
            for lev, s in enumerate(shifts):
                bC, bN = bt[cur], bt[1 - cur]
                tb = scr.tile([128, C4, 64], f32, tag="tb", name="tb")
                nc.vector.tensor_mul(A_(tb, -s), A_(tl[lev], -s), A_(bC, -s))
                m2 = scr.tile([128, C4, 32], f32, tag="m2", name="m2")
                nc.vector.tensor_mul(m2, ql[lev], A_(bC, s))
                nc.vector.tensor_add(A_(bN), A_(bC), A_(tb, -s))
                nc.vector.tensor_add(A_(bN), A_(bN), m2)
                cur = 1 - cur
            nc.vector.tensor_mul(A_(x0p), A_(bt[1]), A_(rt_))

            # ---- x0-dependent outputs (out_p) ----
            x0e = main.tile([128, C4, 64], f32)
            xv0 = x0e.rearrange("p c (i d) -> p c i d", d=2)
            nc.scalar.activation(xv0[:, :, :, 0], A_(x0p), AF.Copy, scale=2.0)
            nc.scalar.activation(xv0[:, :, :, 1], A_(x0p), AF.Copy, scale=2.0)
            Bt1 = main.tile([128, C4, 64], f32)   # x0*u + x1*w
            nc.vector.tensor_mul(Bt1, x0e, ut)
            nc.vector.tensor_add(Bt1, Bt1, bs_)
            drB = main.tile([128, C4, 64], f32)
            drx(drB, Bt1)
            nc.vector.tensor_add(outt[:, :, 64:128], drB, gbm)
            nc.sync.dma_start(out=outv[:, :, 64:128], in_=outt[:, :, 64:128])

    nc.compile()
    return nc


def host_inputs(inputs):
    """Host-side prep: per-core input maps (weights replicated, z sharded)."""
    f = lambda x: np.ascontiguousarray(np.asarray(x, np.float32))
    z = f(inputs["z"])
    W0, W1, W2, W3 = f(inputs["W0"]), f(inputs["W1"]), f(inputs["W2"]), f(inputs["W3"])

    wpk = np.zeros((128, WTOT), np.float32)
    wpk[0:64, OW0 : OW0 + 256] = W0
    for k in range(2):
        sl = slice(k * 128, (k + 1) * 128)
        wpk[:, OW1 + 256 * k : OW1 + 256 * (k + 1)] = W1[sl]
        wpk[:, OW2 + 256 * k : OW2 + 256 * (k + 1)] = W2[sl]
        wpk[:, OW1T + 256 * k : OW1T + 256 * (k + 1)] = W1.T[sl]
        wpk[:, OW2T + 256 * k : OW2T + 256 * (k + 1)] = W2.T[sl]
        wpk[:, OW0T + 64 * k : OW0T + 64 * (k + 1)] = W0.T[sl]

    inv = np.exp(-f(inputs["m_params"])[:, 0])
    invm64 = np.repeat(inv, 2)
    jnv = np.empty(32, np.float32)
    jnv[0] = inv[0]
    jnv[1:] = inv[:-1] + inv[1:]
    eco = (-4.0 * inv).astype(np.float32)   # sigma_c * 4 * inv_c, c>=1
    eco[0] = 4.0 * inv[0]
    eco[31] = 0.0
    row = np.zeros(CTOT, np.float32)
    row[OIV : OIV + 256] = np.tile(invm64, 4)
    row[OJN : OJN + 128] = np.tile(4.0 * jnv, 4)
    row[OFC : OFC + 128] = np.tile(-eco, 4)   # f = -e
    row[OEC : OEC + 128] = np.tile(eco, 4)
    cpk = np.broadcast_to(row, (128, CTOT)).copy()
    for off, b in ((OB0, inputs["b0"]), (OB1, inputs["b1"]), (OB2, inputs["b2"])):
        cpk[:, off : off + 2] = f(b).reshape(2, 128).T
    cpk[:, OW3 : OW3 + 2] = W3[:, 0].reshape(2, 128).T
    cpk = np.ascontiguousarray(cpk)

    shared = {"wpk": np.ascontiguousarray(wpk), "cpk": cpk}
    return [
        {**shared, "z": np.ascontiguousarray(z[i * BSL : (i + 1) * BSL])}
        for i in range(NCORES)
    ]


TRACE = False       # set by dev harnesses to capture an NTFF profile
TMPDIR = None       # set by dev harnesses to keep the trace artifacts
LAST_RESULT = None  # BassKernelResults of the most recent run


def kernel(**inputs) -> np.ndarray:
    global LAST_RESULT
    from concourse.bass_utils import run_bass_kernel_spmd

    nc = build_program()
    in_maps = host_inputs(inputs)
    res = run_bass_kernel_spmd(
        nc, in_maps, list(range(NCORES)), trace=TRACE, tmpdir=TMPDIR
    )
    LAST_RESULT = res
    return np.concatenate([res.results[i]["out"] for i in range(NCORES)], axis=0)
